# revision 63
# baseline (speedup 1.0000x reference)
"""TRN2 Bass kernel for nn_Attention_41506563948971.

Reference computation (per batch b):
    G  = (q @ w + b) @ a^T          [Lq, La]
    P  = softmax(G, axis=q)         (softmax over dim=1, the q axis)
    out= P^T @ q                    [La, H]

Sharding: data-parallel over batch B=8 across the 8 NeuronCores; w, b
replicated. Each core computes one full batch; no collectives.

Numerics: the logits G have sigma ~= 1024 (q,a ~ N(0,1), H=1024), so the
dim-q softmax is extremely peaked and logit errors translate into output
errors on columns whose top-2 gap is small. MM1/MM2 need ~14-bit operand
precision. Instead of 3-pass fp16 hi/lo splits (hi*hi + hi*lo + lo*hi at
1 cycle/row each), each matmul runs as ONE fp16 hi*hi pass plus TWO fp8
DoubleRow cross-correction passes: the lo-side operand is upscaled into
e4m3 and the hi-side downscaled into e5m2 with an exactly-cancelling
scale pair (s, 1/s), so the fp8 products accumulate raw into the same
PSUM group as the fp16 pass. DoubleRow contracts 256 rows per
instruction at 0.5 cycles/row, making each correction pass 1/4 the cost
of an fp16 pass: 1.5 pass-units per matmul instead of 3. Measured
end-to-end relative error ~8e-4 (vs 2.1e-4 for 3-pass fp16), well
inside the 2e-2 budget; each fp8 operand pair is scale-placed so e4m3
stays in its normal range and e5m2 (wide exponent) absorbs the tiny
quotient operand.

MM3's operands are one-hot-ish softmax weights and q, where 11-bit fp16
rounding gives ~2e-4 relative error at full 1-cycle/row PE speed. The
softmax normalization (1/sum) is folded into a per-partition scale of
the small MM3 output, so the big exp matrix is never divided.

Schedule notes (cost-model span ~429 us/core vs 642 us for 3-pass fp16):
- ~44 warmup matmuls fill the startup load-wait and pre-ramp the HAM
  clock gate so the real matmuls start at 2.4 GHz, not 1.2 GHz; the
  startup q loads alternate the SP and GpSimd DMA queues so the 2-deep
  staging rotation never blocks a DMA queue.
- chunk 0 runs six fp16-only passes before its first fp8 cross pass,
  giving the w/q fp8 derivations (which trail w_hi by ~10 us) time to
  land without stalling the PE.
- q^T / a^T transposes go through the DMA xbar transpose engine on the
  SP queue: a DMA issuance blocks its whole engine queue until the
  input is ready, and on ACT that wait would sit ahead of the epilogue
  PSUM drains (PSUM back-pressure stalls the PE). E^T transposes stay
  on ACT where their exp inputs are produced by the same queue. The
  startup q-chunks use PE-transposes instead (their sources arrive
  before the xbar would be free).
- w_hi copies, bias-add, hi-extract, and the output 1/sum scale run on
  ScalarE; the fp32 lo-extract and the e4m3 lo-operand converts run on
  GpSimd (keeping the DVE split/evac/e5m2 chain short); splits and
  e5m2 converts run on the DVE. a-tile 0's operands are persistent and
  prepped during late MM1 so phase 2 starts without a pool-swap stall.
  NOTE: tc.high_priority() hints on the prep chains improve the
  cost-model span by ~13 us but cause a NONDETERMINISTIC NaN race on
  real hardware (per-core failures varying run to run) — do not
  reintroduce them without hardware re-validation.
- MM2's first a-tile runs its four fp16 passes before any cross pass
  (the last qwt8 epilogue converts are still draining out of MM1);
  later iterations run nq-outer so each GT chunk's reduce_max overlaps the next
  chunk's matmuls; MM3 is software-pipelined one a-tile behind.
"""

import sys

sys.path.insert(0, "/opt/trn_rl_repo")

from contextlib import ExitStack

import numpy as np

import concourse.bass as bass
import concourse.bacc as bacc
import concourse.mybir as mybir
import concourse.tile as tile
from concourse.masks import make_identity

dt = mybir.dt
AF = mybir.ActivationFunctionType
OP = mybir.AluOpType
AX = mybir.AxisListType
PM = mybir.MatmulPerfMode

P = 128
H = 1024
KO = H // P          # 8 contraction chunks
KP = KO // 2         # 4 DoubleRow chunk-pairs
LQ = 2048
LA = 2048
NQT = LQ // P        # 16 q row-tiles
NAT = LA // P        # 16 a row-tiles
QC = 512             # free-dim chunk (one fp32 PSUM bank)
NQC = LQ // QC       # 4
HN = QC // 2         # 256: DoubleRow moving-free half
B = 8                # batch == number of cores

SPLIT_DT = dt.float16
E4 = dt.float8e4
E5 = dt.float8e5

# fp8 cross-pass scales (e4m3 gets lo*S, e5m2 gets hi/S; products cancel)
S_WLO = 1024.0       # MM1 cross A: e4m3(w_lo*S) x e5m2(q_hi/S)
S_QLO = 512.0        # MM1 cross B: e4m3(q_lo*S) x e5m2(w_hi/S)
S_ALO = 2048.0       # MM2 cross A: e4m3(a_lo*S) x e5m2(qw/S)
S_QWLO = 64.0        # MM2 cross B: e4m3(qw_lo*S) x e5m2(a_hi/S)


def _split16(nc, pool, src_f32, tag):
    """Split an fp32 tile into (hi, lo) fp16: hi = f16(x), lo = f16(x - hi)."""
    shape = list(src_f32.shape)
    hi = pool.tile(shape, SPLIT_DT, tag=f"{tag}_hi")
    lo = pool.tile(shape, SPLIT_DT, tag=f"{tag}_lo")
    nc.vector.tensor_copy(hi[:], src_f32[:])
    nc.vector.tensor_tensor(lo[:], src_f32[:], hi[:], OP.subtract)
    return hi, lo


def _dr_cross(nc, acc, lhs8, rhs8, m=0, qoff=0, stop_last=False):
    """Accumulate one fp8 DoubleRow cross pass into acc [P, QC].

    lhs8: stationary tile [P, KO, *] (m picks the 128-col block);
    rhs8: moving tile [P, KO, *] (qoff picks the QC-wide q-window).
    4 chunk-pairs x 2 halves at 0.5 cycles/row.
    """
    for t in range(KP):
        lv = lhs8[:, 2 * t:2 * t + 2, m * P:(m + 1) * P]
        for h in range(2):
            last = stop_last and t == KP - 1 and h == 1
            nc.tensor.matmul(
                acc[:, h * HN:(h + 1) * HN],
                lv,
                rhs8[:, 2 * t:2 * t + 2, qoff + h * HN:qoff + (h + 1) * HN],
                start=False,
                stop=last,
                perf_mode=PM.DoubleRow,
            )


def _trace_kernel(tc, q_d, a_d, w_d, b_d, o_d):
    nc = tc.nc
    with ExitStack() as ctx:
        pp = ctx.enter_context(tc.tile_pool(name="persist", bufs=1))
        ps_pool = ctx.enter_context(tc.tile_pool(name="ps", bufs=6, space="PSUM"))
        scratch = ctx.enter_context(tc.tile_pool(name="scratch", bufs=2, space="PSUM"))
        tp_pool = scratch
        op_pool = scratch

        id_sp = pp.tile([P, P], SPLIT_DT, tag="id_sp")
        make_identity(nc, id_sp[:])

        # PE clock warmup: the HAM gate holds the PE at 1.2 GHz until it
        # sees ~3.4 us of sustained activity.
        warm_sb = pp.tile([P, P], SPLIT_DT, tag="warm_sb")
        nc.vector.memset(warm_sb[:], 1.0)
        warm_ps = op_pool.tile([P, P], dt.float32, tag="tp", name="warm_ps")
        NWARM = 44
        for j in range(NWARM):
            nc.tensor.matmul(
                warm_ps[:], warm_sb[:], warm_sb[:],
                start=(j == 0), stop=(j == NWARM - 1),
            )

        b_sb = pp.tile([P, KO], dt.float32, tag="b_sb")

        # QwT = (q @ w + b)^T in [h, q] layout: fp16 hi + fp8 cross operands
        qwt_hi = pp.tile([P, KO, LQ], SPLIT_DT, tag="qwt_hi")
        qwt8_hi = pp.tile([P, KO, LQ], E5, tag="qwt8_hi")   # e5m2(qw / S_ALO)
        qwt8_lo = pp.tile([P, KO, LQ], E4, tag="qwt8_lo")   # e4m3(qw_lo * S_QWLO)
        # q in natural [q, h] layout, rounded to fp16 for MM3.
        q_r = pp.tile([P, NQT, H], dt.float16, tag="q_r")

        # a-tile 0 operands, persistent so its prep can overlap late MM1
        # (phase-2 pools reuse phase-1's SBUF and would serialize otherwise)
        a0_hi = pp.tile([P, KO, P], SPLIT_DT, tag="a0_hi")
        a08_lo = pp.tile([P, KO, P], E4, tag="a08_lo")
        a08_hi = pp.tile([P, KO, P], E5, tag="a08_hi")

        # ---------------- Phase 1: MM1 -> QwT hi/lo ----------------
        with ExitStack() as p1:
            wpool = p1.enter_context(tc.tile_pool(name="wpool", bufs=1))
            stage = p1.enter_context(tc.tile_pool(name="stage", bufs=2))
            split = p1.enter_context(tc.tile_pool(name="split", bufs=2))
            qtp = p1.enter_context(tc.tile_pool(name="qtp", bufs=2))
            epi = p1.enter_context(tc.tile_pool(name="epi", bufs=2))

            w_hi = wpool.tile([P, KO, H], SPLIT_DT, tag="w_hi")
            w8_lo = wpool.tile([P, KO, H], E4, tag="w8_lo")   # e4m3(w_lo*S_WLO)
            w8_hi = wpool.tile([P, KO, H], E5, tag="w8_hi")   # e5m2(w_hi/S_QLO)

            def load_w(k):
                wt = stage.tile([P, H], dt.float32, tag="wstage", name=f"wt{k}")
                nc.sync.dma_start(wt[:], w_d[k * P:(k + 1) * P, :])
                # hi copy on ACT (paces the first fp16 k-loops); lo + fp8
                # derivations on DVE/ACT
                nc.scalar.copy(w_hi[:, k], wt[:])
                wlo = split.tile([P, H], SPLIT_DT, tag="wlo", name=f"wlo{k}")
                nc.vector.tensor_tensor(wlo[:], wt[:], w_hi[:, k], OP.subtract)
                nc.vector.tensor_scalar_mul(w8_lo[:, k], wlo[:], S_WLO)
                nc.scalar.activation(w8_hi[:, k], wt[:], AF.Identity,
                                     scale=1.0 / S_QLO)

            def alloc_qt(qc):
                qt_hi = qtp.tile([P, KO, QC], SPLIT_DT, tag="qt_hi",
                                 name=f"qth{qc}")
                qt8_hi = qtp.tile([P, KO, QC], E5, tag="qt8_hi",
                                  name=f"qt8h{qc}")
                qt8_lo = qtp.tile([P, KO, QC], E4, tag="qt8_lo",
                                  name=f"qt8l{qc}")
                return qt_hi, qt8_hi, qt8_lo

            def prep_q_tile(qc, t, qt, use_pe=False, ldq=None):
                qt_hi, qt8_hi, qt8_lo = qt
                qs = stage.tile([P, H], dt.float32, tag="qstage",
                                name=f"qs{qc}_{t}")
                row0 = qc * QC + t * P
                (ldq or nc.sync).dma_start(qs[:], q_d[row0:row0 + P, :])
                qhi, qlo = _split16(nc, split, qs, "sp")
                nc.scalar.copy(q_r[:, qc * (QC // P) + t], qs[:])
                hi_dst = qt_hi[:, :, t * P:(t + 1) * P]
                lo_dst = split.tile([P, KO, P], SPLIT_DT, tag="tlo",
                                    name=f"tlo{qc}_{t}")
                if use_pe:
                    # PE transposes, batched 8 per PSUM bank, DVE evacuation
                    for src, dst in ((qhi, hi_dst), (qlo, lo_dst[:])):
                        tp = tp_pool.tile([P, KO * P], SPLIT_DT, tag="tp")
                        for k in range(KO):
                            nc.tensor.transpose(
                                tp[:, k * P:(k + 1) * P],
                                src[:, k * P:(k + 1) * P],
                                id_sp[:],
                            )
                        nc.vector.tensor_copy(
                            dst, tp[:].rearrange("p (k c) -> p k c", k=KO)
                        )
                else:
                    # xbar DMA transpose on the SP queue: its issuance waits
                    # for the split, and on ACT that wait would block the
                    # epilogue drains queued behind it (PSUM back-pressure)
                    nc.sync.dma_start_transpose(hi_dst, qhi[:])
                    nc.sync.dma_start_transpose(lo_dst[:], qlo[:])
                # fp8 operand converts from the transposed fp16 tiles
                nc.vector.tensor_scalar_mul(
                    qt8_hi[:, :, t * P:(t + 1) * P], hi_dst, 1.0 / S_WLO)
                nc.vector.tensor_scalar_mul(
                    qt8_lo[:, :, t * P:(t + 1) * P], lo_dst[:], S_QLO)

            def prep_a0_tile():
                # a-tile 0 prep, emitted during late MM1 so phase 2 starts hot
                at = stage.tile([P, H], dt.float32, tag="qstage", name="at0")
                nc.sync.dma_start(at[:], a_d[0:P, :])
                a_hi, a_lo = _split16(nc, split, at, "sp")
                alo_t = split.tile([P, KO, P], SPLIT_DT, tag="tlo", name="atl0")
                nc.sync.dma_start_transpose(a0_hi[:], a_hi[:])
                nc.sync.dma_start_transpose(alo_t[:], a_lo[:])
                nc.vector.tensor_scalar_mul(a08_lo[:], alo_t[:], S_ALO)
                nc.vector.tensor_scalar_mul(a08_hi[:], a0_hi[:], 1.0 / S_QWLO)

            # q-chunk 0's loads/splits/transposes first so PE starts
            # immediately (loads alternate SP/GpSimd DMA queues so the
            # 2-deep staging rotation never blocks a queue); w loads
            # overlap the transposes.
            nc.gpsimd.dma_start(b_sb[:], b_d.rearrange("(m p) -> p m", p=P))
            qt_cur = alloc_qt(0)
            for t in range(QC // P):
                prep_q_tile(0, t, qt_cur, use_pe=True,
                            ldq=nc.gpsimd if t % 2 else nc.sync)
            for k in range(KO):
                load_w(k)

            def fp16_pass(qt_hi, m, name):
                acc = ps_pool.tile([P, QC], dt.float32, tag="ps", name=name)
                for k in range(KO):
                    nc.tensor.matmul(
                        acc[:],
                        w_hi[:, k, m * P:(m + 1) * P],
                        qt_hi[:, k, :],
                        start=(k == 0),
                        stop=False,
                    )
                return acc

            def epilogue(acc, qc, m):
                # bias add + hi extract (ACT), fp32 lo subtract (GpSimd),
                # fp8 converts (DVE)
                qwf = epi.tile([P, QC], dt.float32, tag="qwf")
                nc.scalar.activation(
                    qwf[:], acc[:], AF.Identity, bias=b_sb[:, m:m + 1]
                )
                dhi = qwt_hi[:, m, qc * QC:(qc + 1) * QC]
                nc.scalar.copy(dhi, qwf[:])
                nc.vector.tensor_scalar_mul(
                    qwt8_hi[:, m, qc * QC:(qc + 1) * QC], qwf[:],
                    1.0 / S_ALO)
                dlo = epi.tile([P, QC], dt.float32, tag="dlo")
                nc.gpsimd.tensor_tensor(dlo[:], qwf[:], dhi, OP.subtract)
                nc.vector.tensor_scalar_mul(
                    qwt8_lo[:, m, qc * QC:(qc + 1) * QC], dlo[:], S_QWLO)

            # chunk 0: six fp16-only passes run first — they need only
            # w_hi + the transposed chunk, giving the w/q fp8 derivations
            # time to land before the first cross pass
            qt_hi, qt8_hi, qt8_lo = qt_cur
            qt_next = alloc_qt(1)
            accs = []
            for m in range(6):
                accs.append(fp16_pass(qt_hi, m, f"acc0_{m}"))
                if m % 2 == 0:
                    prep_q_tile(1, m // 2, qt_next, use_pe=True)
            for m in range(KO):
                if m >= 6:
                    accs.append(fp16_pass(qt_hi, m, f"acc0_{m}"))
                _dr_cross(nc, accs[m], w8_lo, qt8_hi, m=m)
                _dr_cross(nc, accs[m], w8_hi, qt8_lo, m=m, stop_last=True)
                epilogue(accs[m], 0, m)
                if m == 0:
                    prep_q_tile(1, 3, qt_next, use_pe=True)
            qt_cur = qt_next

            for qc in range(1, NQC):
                qt_hi, qt8_hi, qt8_lo = qt_cur
                if qc + 1 < NQC:
                    qt_next = alloc_qt(qc + 1)
                for m in range(KO):
                    acc = fp16_pass(qt_hi, m, f"acc{qc}_{m}")
                    _dr_cross(nc, acc, w8_lo, qt8_hi, m=m)
                    _dr_cross(nc, acc, w8_hi, qt8_lo, m=m, stop_last=True)
                    epilogue(acc, qc, m)
                    # interleave the next chunk's per-tile prep between
                    # m-blocks
                    if qc + 1 < NQC and m < QC // P:
                        prep_q_tile(qc + 1, m, qt_next)
                    if qc == NQC - 1 and m == 0:
                        prep_a0_tile()
                if qc + 1 < NQC:
                    qt_cur = qt_next

        # ---------------- Phase 2: MM2 + softmax + MM3 ----------------
        with ExitStack() as p2:
            astage = p2.enter_context(tc.tile_pool(name="astage", bufs=4))
            asplit = p2.enter_context(tc.tile_pool(name="asplit", bufs=3))
            atp = p2.enter_context(tc.tile_pool(name="atp", bufs=2))
            ppool = p2.enter_context(tc.tile_pool(name="ppool", bufs=2))
            ptpool = p2.enter_context(tc.tile_pool(name="ptpool", bufs=2))
            outp = p2.enter_context(tc.tile_pool(name="outp", bufs=2))
            redp = p2.enter_context(tc.tile_pool(name="redp", bufs=4))

            def prep_a_tile(i):
                at = astage.tile([P, H], dt.float32, tag="astage", name=f"at{i}")
                nc.sync.dma_start(at[:], a_d[i * P:(i + 1) * P, :])
                a_hi, a_lo = _split16(nc, asplit, at, "asp")
                at_hi = atp.tile([P, KO, P], SPLIT_DT, tag="at_hi", name=f"ath{i}")
                alo_t = atp.tile([P, KO, P], SPLIT_DT, tag="alo_t", name=f"atl{i}")
                nc.sync.dma_start_transpose(at_hi[:], a_hi[:])
                nc.sync.dma_start_transpose(alo_t[:], a_lo[:])
                at8_lo = atp.tile([P, KO, P], E4, tag="at8_lo", name=f"a8l{i}")
                at8_hi = atp.tile([P, KO, P], E5, tag="at8_hi", name=f"a8h{i}")
                nc.vector.tensor_scalar_mul(at8_lo[:], alo_t[:], S_ALO)
                nc.vector.tensor_scalar_mul(at8_hi[:], at_hi[:], 1.0 / S_QWLO)
                return at_hi, at8_lo, at8_hi

            def do_mm3(pt_sb, rinv, i):
                # MM3: out[a, h] = sum_q ET[q, a] * q[q, h], then * (1/sum)
                o_sb = outp.tile([P, H], dt.float32, tag="o_sb", name=f"osb{i}")
                for nh in range(H // QC):
                    acc = op_pool.tile([P, QC], dt.float32, tag="tp")
                    for t in range(NQT):
                        nc.tensor.matmul(
                            acc[:],
                            pt_sb[:, t, :],
                            q_r[:, t, nh * QC:(nh + 1) * QC],
                            start=(t == 0),
                            stop=(t == NQT - 1),
                        )
                    nc.scalar.activation(
                        o_sb[:, nh * QC:(nh + 1) * QC], acc[:], AF.Identity,
                        scale=rinv[:],
                    )
                nc.sync.dma_start(o_d[i * P:(i + 1) * P, :], o_sb[:])

            at_cur = (a0_hi, a08_lo, a08_hi)  # prepped during late MM1
            mm3_prev = None

            for i in range(NAT):
                at_hi, at8_lo, at8_hi = at_cur

                # next a-tile prep first: its split/transpose/convert chain
                # rides ahead of this iteration's reductions on DVE/SP, so
                # MM2(i+1) never waits on operands
                if i + 1 < NAT:
                    at_next = prep_a_tile(i + 1)

                # MM2 nq-outer: each GT chunk finishes early so its
                # reduce_max overlaps the next chunk's matmuls.
                gt = []
                gmax = redp.tile([P, NQC], dt.float32, tag="gmax")

                def mm2_fp16(nq):
                    g = ps_pool.tile([P, QC], dt.float32, tag="ps",
                                     name=f"gt{nq}")
                    for k in range(KO):
                        nc.tensor.matmul(
                            g[:],
                            at_hi[:, k, :],
                            qwt_hi[:, k, nq * QC:(nq + 1) * QC],
                            start=(k == 0),
                            stop=False,
                        )
                    return g

                def mm2_cross(g, nq):
                    _dr_cross(nc, g, at8_lo, qwt8_hi, qoff=nq * QC)
                    _dr_cross(nc, g, at8_hi, qwt8_lo, qoff=nq * QC,
                              stop_last=True)
                    nc.vector.reduce_max(gmax[:, nq:nq + 1], g[:], axis=AX.X)

                if i == 0:
                    # fp16 passes first: the last qwt8 epilogue converts are
                    # still draining out of MM1 when MM2 starts
                    gt = [mm2_fp16(nq) for nq in range(NQC)]
                    for nq in range(NQC):
                        mm2_cross(gt[nq], nq)
                else:
                    for nq in range(NQC):
                        g = mm2_fp16(nq)
                        mm2_cross(g, nq)
                        gt.append(g)

                negm = redp.tile([P, 1], dt.float32, tag="negm")
                nc.vector.reduce_max(negm[:], gmax[:], axis=AX.X, negate=True)

                # exps first so they're ahead of MM3's scales on ACT's
                # in-order queue
                p_sb = ppool.tile([P, LQ], dt.float16, tag="p_sb")
                sums = redp.tile([P, NQC], dt.float32, tag="sums")
                for nq in range(NQC):
                    nc.scalar.activation(
                        p_sb[:, nq * QC:(nq + 1) * QC],
                        gt[nq][:],
                        AF.Exp,
                        bias=negm[:],
                        scale=1.0,
                        accum_out=sums[:, nq:nq + 1],
                    )
                sall = redp.tile([P, 1], dt.float32, tag="sall")
                nc.vector.reduce_sum(sall[:], sums[:], axis=AX.X)
                rinv = redp.tile([P, 1], dt.float32, tag="rinv")
                nc.vector.reciprocal(rinv[:], sall[:])

                # PE work that needs no softmax results fills the window
                # while ACT runs the exps: the previous iteration's MM3.
                if mm3_prev is not None:
                    do_mm3(*mm3_prev)

                # transpose E=[a,q] -> ET=[q,a] via xbar DMA, per chunk
                pt_sb = ptpool.tile([P, NQT, P], dt.float16, tag="pt_sb")
                for nq in range(NQC):
                    nc.scalar.dma_start_transpose(
                        pt_sb[:, nq * NQC:(nq + 1) * NQC, :],
                        p_sb[:, nq * QC:(nq + 1) * QC],
                    )

                mm3_prev = (pt_sb, rinv, i)
                if i + 1 < NAT:
                    at_cur = at_next

            do_mm3(*mm3_prev)


_CACHE = {}


def build_nc():
    if "nc" in _CACHE:
        return _CACHE["nc"]
    nc = bacc.Bacc("TRN2", target_bir_lowering=False, debug=False)
    q_d = nc.dram_tensor("q", [LQ, H], dt.float32, kind="ExternalInput").ap()
    a_d = nc.dram_tensor("a", [LA, H], dt.float32, kind="ExternalInput").ap()
    w_d = nc.dram_tensor("w", [H, H], dt.float32, kind="ExternalInput").ap()
    b_d = nc.dram_tensor("b", [H], dt.float32, kind="ExternalInput").ap()
    o_d = nc.dram_tensor("o", [LA, H], dt.float32, kind="ExternalOutput").ap()
    with tile.TileContext(nc) as tc:
        _trace_kernel(tc, q_d, a_d, w_d, b_d, o_d)
    nc.compile()
    _CACHE["nc"] = nc
    return nc


def get_runner():
    """Build (once) a cached jitted SPMD executable over the 8 cores."""
    if "runner" in _CACHE:
        return _CACHE["runner"]
    import jax
    from jax.sharding import Mesh, PartitionSpec
    from jax.experimental.shard_map import shard_map

    from concourse import bass2jax

    nc = build_nc()
    bass2jax.install_neuronx_cc_hook()

    partition_name = nc.partition_id_tensor.name if nc.partition_id_tensor else None
    in_names, out_names, out_avals, zero_outs = [], [], [], []
    for alloc in nc.m.functions[0].allocations:
        if not isinstance(alloc, mybir.MemoryLocationSet):
            continue
        name = alloc.memorylocations[0].name
        if alloc.kind == "ExternalInput":
            if name != partition_name:
                in_names.append(name)
        elif alloc.kind == "ExternalOutput":
            shape = tuple(alloc.tensor_shape)
            dtype = mybir.dt.np(alloc.dtype)
            out_names.append(name)
            out_avals.append(jax.core.ShapedArray(shape, dtype))
            zero_outs.append(np.zeros(shape, dtype))
    n_params = len(in_names)
    all_in_names = list(in_names) + list(out_names)
    if partition_name is not None:
        all_in_names.append(partition_name)

    def _body(*args):
        operands = list(args)
        if partition_name is not None:
            operands.append(bass2jax.partition_id_tensor())
        outs = bass2jax._bass_exec_p.bind(
            *operands,
            out_avals=tuple(out_avals),
            in_names=tuple(all_in_names),
            out_names=tuple(out_names),
            lowering_input_output_aliases=(),
            sim_require_finite=True,
            sim_require_nnan=True,
            nc=nc,
        )
        return tuple(outs)

    devices = jax.devices()[:B]
    mesh = Mesh(np.asarray(devices), ("core",))
    n_outs = len(out_names)
    in_specs = (PartitionSpec("core"),) * (n_params + n_outs)
    out_specs = (PartitionSpec("core"),) * n_outs
    sharded = jax.jit(
        shard_map(
            _body, mesh=mesh, in_specs=in_specs, out_specs=out_specs, check_rep=False
        ),
        keep_unused=True,
    )
    runner = (sharded, in_names, out_names, out_avals, zero_outs)
    _CACHE["runner"] = runner
    return runner


def run_cores(in_maps):
    """Run the kernel SPMD over 8 cores; in_maps is a list of 8 dicts."""
    sharded, in_names, out_names, out_avals, zero_outs = get_runner()
    concat_in = [
        np.concatenate([np.asarray(m[name]) for m in in_maps], axis=0)
        for name in in_names
    ]
    concat_zeros = [
        np.zeros((B * z.shape[0], *z.shape[1:]), z.dtype) for z in zero_outs
    ]
    out_arrs = sharded(*concat_in, *concat_zeros)
    return [
        {
            name: np.asarray(out_arrs[j]).reshape(B, *out_avals[j].shape)[c]
            for j, name in enumerate(out_names)
        }
        for c in range(B)
    ]


def kernel(q, a, w, b):
    q = np.ascontiguousarray(np.asarray(q, dtype=np.float32))
    a = np.ascontiguousarray(np.asarray(a, dtype=np.float32))
    w = np.ascontiguousarray(np.asarray(w, dtype=np.float32))
    b = np.ascontiguousarray(np.asarray(b, dtype=np.float32))
    assert q.shape == (B, LQ, H) and a.shape == (B, LA, H)
    assert w.shape == (H, H) and b.shape == (H,)

    in_maps = [{"q": q[i], "a": a[i], "w": w, "b": b} for i in range(B)]
    try:
        from concourse.bass_utils import run_bass_kernel_spmd

        results = run_bass_kernel_spmd(
            build_nc(), in_maps, core_ids=list(range(B))
        ).results
    except Exception:
        # fallback: cached jitted shard_map runner (same execution path)
        results = run_cores(in_maps)
    return np.stack([results[i]["o"] for i in range(B)], axis=0)
